# revision 8
# baseline (speedup 1.0000x reference)
"""Trainium2 Bass kernel for nn_LEIterator (CG tensor-product iterator).

Layout/sharding: 8 cores = 2 sample-halves (128 samples on SBUF partitions)
x 4 k-groups (each core computes CG combination slots k in {2g, 2g+1}).
All gather indices are compile-time constants (seeded rng), so the per-core
gathers are done host-side into tiny pre-gathered input tensors; the device
program is identical on every core (pure SPMD).

The kernel is output-write bound: outputs are written as bf16 (rel-err
~2e-3, well under the 2e-2 gate) and widened to f32 on the host — 21.7 MB
of HBM writes per core at the ~400 GB/s measured write rate (~56 us DMA
busy). The nu=3 expansion (tmp[s, ab] * w[s, c], per-partition scalar, one
instruction per c) is split across three engines so compute stays under
the DMA floor: DVE tensor_scalar (2x mode, ~194 ns/op) takes 12 blocks +
all u(x)v tmp products, ScalarE activation-with-scale-AP takes 5 blocks,
GPSIMD takes the nu=2 outer products + 3 blocks. The device-side nu3
block layout is [c, ab] (c outer) so every op writes one dense 256-element
bf16 run; the host unshuffle transposes back.

This version is RAW bass (no TileContext): hand-wired semaphores instead
of the tile framework, which removes the ~7.5 us tile-entry barrier
preamble and lets the input DMA run at t~0. Semaphores are cleared at the
END of the program so repeated executions of the NEFF start from zero.
Sync-engine DMA issues are ordered by expected block readiness so the DMA
queue runs gapless from ~6 us onward.
"""

import numpy as np

import concourse.bass as bass
import concourse.mybir as mybir
from concourse.bass_utils import run_bass_kernel_spmd

K = 8        # CG m-combinations kept per l_tuple
Q = 16       # radial channels
S = 256      # samples
L_MAX = 2
HALF = 128   # samples per core (S / 2 halves)
NU2_TUPLES = 6
NU3_TUPLES = 10
NU2_BLOCKS = NU2_TUPLES * 2   # per-core: 2 k-slots per tuple
NU3_BLOCKS = NU3_TUPLES * 2
QA0 = 0
QB0 = QA0 + NU2_BLOCKS * Q
PU0 = QB0 + NU2_BLOCKS * Q
PV0 = PU0 + NU3_BLOCKS * Q
PW0 = PV0 + NU3_BLOCKS * Q
INP_W = PW0 + NU3_BLOCKS * Q
ROWS2 = NU2_TUPLES * K * Q * Q          # 12288 rows in full output
ROWS3 = NU3_TUPLES * K * Q * Q * Q      # 327680
TOTAL_ROWS = ROWS2 + ROWS3              # 339968
BW = Q * Q * Q                          # 4096 cols per nu3 block


def _build_structure():
    """Exact replica of reference._build_structure's rng call sequence."""
    rng = np.random.default_rng(0)
    t2 = []
    for l1 in range(L_MAX + 1):
        for l2 in range(l1, L_MAX + 1):
            ip = rng.integers(0, 2 * l1 + 1, K)
            i1 = rng.integers(0, 2 * l2 + 1, K)
            mult = (rng.random(K) + 0.5).astype(np.float32)
            t2.append(((l1, l2), ip, i1, mult))
    t3 = []
    for l1 in range(L_MAX + 1):
        for l2 in range(l1, L_MAX + 1):
            for l3 in range(l2, L_MAX + 1):
                ip = rng.integers(0, K, K)
                i1 = rng.integers(0, 2 * l3 + 1, K)
                mult = (rng.random(K) + 0.5).astype(np.float32)
                t3.append(((l1, l2, l3), ip, i1, mult))
    return t2, t3


_T2, _T3 = _build_structure()
_S2MAP = {lt: (ip, i1) for lt, ip, i1, _ in _T2}

_NC = None


def _build_program():
    f32 = mybir.dt.float32
    bf16 = mybir.dt.bfloat16
    MULT = mybir.AluOpType.mult
    COPY = mybir.ActivationFunctionType.Copy
    nc = bass.Bass("TRN2")

    inp = nc.dram_tensor("inp", [HALF, INP_W], f32, kind="ExternalInput")
    out2 = nc.dram_tensor("out2", [HALF, NU2_BLOCKS * Q * Q], bf16, kind="ExternalOutput")
    out3 = nc.dram_tensor("out3", [HALF, NU3_BLOCKS * BW], bf16, kind="ExternalOutput")

    tinp = nc.alloc_sbuf_tensor("tinp", [HALF, INP_W], f32)
    ttmp = nc.alloc_sbuf_tensor("ttmp", [HALF, NU3_BLOCKS * Q * Q], bf16)
    t3s = nc.alloc_sbuf_tensor("t3s", [HALF, NU3_BLOCKS * BW], bf16)
    t2s = nc.alloc_sbuf_tensor("t2s", [HALF, NU2_BLOCKS * Q * Q], bf16)
    tact_warm = nc.alloc_sbuf_tensor("tact_warm", [HALF, 1], bf16)

    inp_sem = nc.alloc_semaphore("inp_sem")    # pu/pv/pw input DMA landed (16)
    inp2_sem = nc.alloc_semaphore("inp2_sem")  # qa/qb input DMA landed (16)
    tmpd_sem = nc.alloc_semaphore("tmpd_sem")  # DVE tmp pairs for ACT/GP (1 per pair)
    dveb_sem = nc.alloc_semaphore("dveb_sem")  # DVE half/full blocks done
    actb_sem = nc.alloc_semaphore("actb_sem")  # ACT blocks done
    dma_done = nc.alloc_semaphore("dma_done")  # all output DMA completions
    all_sems = [inp_sem, inp2_sem, tmpd_sem, dveb_sem, actb_sem, dma_done]

    # split input DMA: pu/pv/pw land first (unblocks tmps+TS), qa/qb second
    nc.sync.dma_start(
        tinp[:, PU0:INP_W], inp[:, PU0:INP_W]
    ).then_inc(inp_sem, 16)
    nc.sync.dma_start(
        tinp[:, QA0:PU0], inp[:, QA0:PU0]
    ).then_inc(inp2_sem, 16)

    tqa = tinp[:, QA0 : QA0 + NU2_BLOCKS * Q]
    tqb = tinp[:, QB0 : QB0 + NU2_BLOCKS * Q]
    tpu = tinp[:, PU0 : PU0 + NU3_BLOCKS * Q]
    tpv = tinp[:, PV0 : PV0 + NU3_BLOCKS * Q]
    tpw = tinp[:, PW0 : PW0 + NU3_BLOCKS * Q]

    # every compute engine gates once on the input DMA
    nc.vector.wait_ge(inp_sem, 16)
    nc.scalar.wait_ge(inp_sem, 16)

    def tmp_pair(pr, inc=False):
        """tmp blocks (2pr, 2pr+1): [p, 2, 16(a), 16(b)] = u (x) v  (DVE)"""
        b0 = 2 * pr
        sl = slice(b0 * Q, (b0 + 2) * Q)
        tmpv = ttmp[:, b0 * Q * Q : (b0 + 2) * Q * Q]
        u = (
            tpu[:, sl]
            .rearrange("p (c a) -> p c a", a=Q)
            .unsqueeze(3)
            .broadcast_to([HALF, 2, Q, Q])
        )
        v = (
            tpv[:, sl]
            .rearrange("p (c b) -> p c b", b=Q)
            .unsqueeze(2)
            .broadcast_to([HALF, 2, Q, Q])
        )
        ins = nc.vector.tensor_tensor(
            tmpv.rearrange("p (c a b) -> p c a b", a=Q, b=Q), u, v, MULT
        )
        if inc:
            ins.then_inc(tmpd_sem, 1)

    def tmp_quad(b0):
        """tmp blocks b0..b0+3 in one TT: [p, 4, 16(a), 16(b)] = u (x) v"""
        sl = slice(b0 * Q, (b0 + 4) * Q)
        tmpv = ttmp[:, b0 * Q * Q : (b0 + 4) * Q * Q]
        u = (
            tpu[:, sl]
            .rearrange("p (c a) -> p c a", a=Q)
            .unsqueeze(3)
            .broadcast_to([HALF, 4, Q, Q])
        )
        v = (
            tpv[:, sl]
            .rearrange("p (c b) -> p c b", b=Q)
            .unsqueeze(2)
            .broadcast_to([HALF, 4, Q, Q])
        )
        nc.vector.tensor_tensor(
            tmpv.rearrange("p (c a b) -> p c a b", a=Q, b=Q), u, v, MULT
        )

    def ts_ops(b, cs, engine, sem=None):
        """block b, [c, ab] layout: out[:, c*256:(c+1)*256] = tmp * w[:, c]"""
        tb = ttmp[:, b * Q * Q : (b + 1) * Q * Q]
        last = None
        for c in cs:
            ov = t3s[:, b * BW + c * Q * Q : b * BW + (c + 1) * Q * Q]
            wc = tpw[:, b * Q + c : b * Q + c + 1]
            if engine == "dve":
                last = nc.vector.tensor_scalar_mul(ov, tb, wc)
            elif engine == "act":
                last = nc.scalar.activation(ov, tb, COPY, bias=0.0, scale=wc)
            else:
                last = nc.gpsimd.tensor_scalar_mul(ov, tb, wc)
        if sem is not None:
            last.then_inc(sem, 1)

    def dve_nu2():
        last = None
        for b in range(0, NU2_BLOCKS, 4):
            sl = slice(b * Q, (b + 4) * Q)
            a = (
                tqa[:, sl]
                .rearrange("p (c a) -> p c a", a=Q)
                .unsqueeze(3)
                .broadcast_to([HALF, 4, Q, Q])
            )
            bb = (
                tqb[:, sl]
                .rearrange("p (c b) -> p c b", b=Q)
                .unsqueeze(2)
                .broadcast_to([HALF, 4, Q, Q])
            )
            o = t2s[:, b * Q * Q : (b + 4) * Q * Q].rearrange(
                "p (c a b) -> p c a b", a=Q, b=Q
            )
            last = nc.vector.tensor_tensor(o, a, bb, MULT)
        last.then_inc(dveb_sem, 1)

    n_dma = [0]  # DMAs that increment dma_done (the input DMA incs inp_sem)

    def dma_o3(b, c0, c1, sem, val):
        nc.sync.wait_ge(sem, val)
        nc.sync.dma_start(
            out3[:, b * BW + c0 * Q * Q : b * BW + c1 * Q * Q],
            t3s[:, b * BW + c0 * Q * Q : b * BW + c1 * Q * Q],
        ).then_inc(dma_done, 16)
        n_dma[0] += 1

    # ---- DVE stream ------------------------------------------------------
    tmp_pair(0)                    # DVE blocks 0,1
    tmp_pair(7, inc=True)          # ACT blocks 14,15   (tmpd=1)
    ts_ops(0, range(0, 8), "dve", dveb_sem)       # dveb=1
    ts_ops(0, range(8, Q), "dve", dveb_sem)       # dveb=2
    tmp_pair(8, inc=True)          # ACT blocks 16,17   (tmpd=2)
    tmp_pair(9, inc=True)          # ACT blocks 18,19   (tmpd=3)
    ts_ops(1, range(Q), "dve", dveb_sem)          # dveb=3
    dve_nu2()                                     # dveb=4
    for pr, (ba, bb_) in enumerate(
        [(2, 3), (4, 5), (6, 7), (8, 9), (10, 11), (12, 13)]
    ):
        tmp_pair(pr + 1)
        ts_ops(ba, range(Q), "dve", dveb_sem)     # dveb=5,7,9,11,13,15
        ts_ops(bb_, range(Q), "dve", dveb_sem)    # dveb=6,8,10,12,14,16

    # ---- ACT stream ------------------------------------------------------
    # dummy 1-col activation to pull the Copy table load off the hot path
    nc.scalar.activation(tact_warm[:], ttmp[:, 0:1], COPY, bias=0.0, scale=1.0)
    nc.scalar.wait_ge(tmpd_sem, 1)
    ts_ops(14, range(Q), "act", actb_sem)         # actb=1
    ts_ops(15, range(Q), "act", actb_sem)         # actb=2
    nc.scalar.wait_ge(tmpd_sem, 2)
    ts_ops(16, range(Q), "act", actb_sem)         # actb=3
    ts_ops(17, range(Q), "act", actb_sem)         # actb=4
    nc.scalar.wait_ge(tmpd_sem, 3)
    ts_ops(18, range(Q), "act", actb_sem)         # actb=5
    ts_ops(19, range(Q), "act", actb_sem)         # actb=6

    # ---- Sync stream: DMA issues in expected readiness order -------------
    dma_o3(0, 0, 8, dveb_sem, 1)
    dma_o3(0, 8, Q, dveb_sem, 2)
    dma_o3(1, 0, Q, dveb_sem, 3)
    dma_o3(14, 0, Q, actb_sem, 1)
    nc.sync.wait_ge(dveb_sem, 4)
    nc.sync.dma_start(out2[:], t2s[:]).then_inc(dma_done, 16)
    n_dma[0] += 1
    dma_o3(2, 0, Q, dveb_sem, 5)
    dma_o3(3, 0, Q, dveb_sem, 6)
    dma_o3(15, 0, Q, actb_sem, 2)
    dma_o3(4, 0, Q, dveb_sem, 7)
    dma_o3(5, 0, Q, dveb_sem, 8)
    dma_o3(16, 0, Q, actb_sem, 3)
    dma_o3(6, 0, Q, dveb_sem, 9)
    dma_o3(7, 0, Q, dveb_sem, 10)
    dma_o3(8, 0, Q, dveb_sem, 11)
    dma_o3(17, 0, Q, actb_sem, 4)
    dma_o3(9, 0, Q, dveb_sem, 12)
    dma_o3(10, 0, Q, dveb_sem, 13)
    dma_o3(18, 0, Q, actb_sem, 5)
    dma_o3(11, 0, Q, dveb_sem, 14)
    dma_o3(12, 0, Q, dveb_sem, 15)
    dma_o3(13, 0, Q, dveb_sem, 16)
    dma_o3(19, 0, Q, actb_sem, 6)

    # ---- drain -----------------------------------------------------------
    nc.sync.wait_ge(dma_done, 16 * n_dma[0])
    nc.all_engine_barrier()
    for s in all_sems:
        nc.gpsimd.sem_clear(s)
    return nc


def _get_nc():
    global _NC
    if _NC is None:
        _NC = _build_program()
    return _NC


def _make_in_maps(LE1):
    in_maps = []
    for c in range(8):
        h, g = divmod(c, 4)
        sl = slice(h * HALF, (h + 1) * HALF)
        buf = np.empty((HALF, INP_W), np.float32)
        qa = buf[:, QA0 : QA0 + NU2_BLOCKS * Q]
        qb = buf[:, QB0 : QB0 + NU2_BLOCKS * Q]
        pu = buf[:, PU0 : PU0 + NU3_BLOCKS * Q]
        pv = buf[:, PV0 : PV0 + NU3_BLOCKS * Q]
        pw = buf[:, PW0 : PW0 + NU3_BLOCKS * Q]
        for ti, ((l1, l2), ip, i1, mult) in enumerate(_T2):
            for j in range(2):
                k = 2 * g + j
                b = ti * 2 + j
                qa[:, b * Q : (b + 1) * Q] = LE1[l1][ip[k], :, sl].T
                qb[:, b * Q : (b + 1) * Q] = LE1[l2][i1[k], :, sl].T * mult[k]
        for ti, ((l1, l2, l3), ip3, i13, mult3) in enumerate(_T3):
            ip2, i12 = _S2MAP[(l1, l2)]
            for j in range(2):
                k = 2 * g + j
                b = ti * 2 + j
                kk = ip3[k]
                pu[:, b * Q : (b + 1) * Q] = LE1[l1][ip2[kk], :, sl].T
                pv[:, b * Q : (b + 1) * Q] = LE1[l2][i12[kk], :, sl].T
                pw[:, b * Q : (b + 1) * Q] = LE1[l3][i13[k], :, sl].T * mult3[k]
        in_maps.append({"inp": buf})
    return in_maps


LAST_RUN = None  # BassKernelResults of the most recent kernel() call (for test.py)
TRACE = False


def kernel(LE1_l0, LE1_l1, LE1_l2):
    global LAST_RUN
    LE1 = {
        0: np.ascontiguousarray(np.asarray(LE1_l0, dtype=np.float32)),
        1: np.ascontiguousarray(np.asarray(LE1_l1, dtype=np.float32)),
        2: np.ascontiguousarray(np.asarray(LE1_l2, dtype=np.float32)),
    }
    nc = _get_nc()
    in_maps = _make_in_maps(LE1)
    LAST_RUN = run_bass_kernel_spmd(
        nc, in_maps, core_ids=list(range(8)), trace=TRACE
    )
    res = LAST_RUN.results

    out = np.empty((TOTAL_ROWS, S), np.float32)
    for core in range(8):
        h, g = divmod(core, 4)
        cs = slice(h * HALF, (h + 1) * HALF)
        # device outputs are bf16; widen exactly via bit shift
        o2 = np.asarray(res[core]["out2"])
        o2 = (o2.view(np.uint16).astype(np.uint32) << 16).view(np.float32)
        o3 = np.asarray(res[core]["out3"])
        o3 = (o3.view(np.uint16).astype(np.uint32) << 16).view(np.float32)
        for ti in range(NU2_TUPLES):
            for j in range(2):
                k = 2 * g + j
                b = ti * 2 + j
                r0 = ti * (K * Q * Q) + k * Q * Q
                out[r0 : r0 + Q * Q, cs] = o2[:, b * Q * Q : (b + 1) * Q * Q].T
        for ti in range(NU3_TUPLES):
            for j in range(2):
                k = 2 * g + j
                b = ti * 2 + j
                r0 = ROWS2 + ti * (K * BW) + k * BW
                # device block is [s, c, f]; reference rows are f*Q + c
                blk = o3[:, b * BW : (b + 1) * BW].reshape(HALF, Q, Q * Q)
                out[r0 : r0 + BW, cs] = blk.transpose(2, 1, 0).reshape(BW, HALF)
    return out
